# revision 38
# baseline (speedup 1.0000x reference)
"""Trainium2 Bass kernel for nn_DecoderBlock (two chained MHA layers, no out-proj).

Sharding: 8 cores = (batch b = core//2) x (head-half g = core%2).
Each core computes 8 heads (512 feature cols) of self-attention for its batch,
normalizes its half of x1 on device, AllGathers the bf16 x1 halves across the
batch pair, then computes cross-attention for its 8 heads (q2 projection
contracts the full gathered x1 against its own-column slice of wq_cross).

All matmul operands are bf16 (fp32 matmuls run in fp32_mode=HIGH, don't keep
the PE HAM clock-gate warm, and cost extra cycles/row; bf16 runs 1 cyc/row at
2.4 GHz with FWL weight loads). PSUM accumulation stays fp32. Device layout is
feature-major: activations live as [d, s] tiles so every matmul contracts over
the partition dim. Softmax runs on scoresT [sk, sq]: exp on the scalar engine
(scale=1/8 free affine + per-partition src-mask bias), causal upper blocks
skipped, diagonal blocks masked by a 0/1 multiply after exp. The AV matmul
uses lhsT=[v_h | 1] so the softmax denominator falls out as row 64. Phase-1
denominators are DMA-packed into one [8, S] tile, reciprocal'd in a single DVE
call, and broadcast per head with a selector matmul; phase-2 output is
returned unnormalized (+denominator row) and normalized on host.
"""

import sys

if '/opt/trn_rl_repo' not in sys.path:
    sys.path.insert(0, '/opt/trn_rl_repo')

import numpy as np

B, S, D, H, DKH = 4, 1024, 1024, 16, 64
NCORES = 8
HPC = H // 2            # 8 heads per core
CPC = HPC * DKH         # 512 feature cols per core
ST = S // 128           # 8 seq tiles
NDT = D // 128          # 8 feature chunks
AUG = DKH + 1           # 65 (v columns + ones)
NA = 6                  # heads in the early (hidden) exchange half
NB = HPC - NA
TA = NA // 2            # x1t tiles in the early half
JLA = tuple(list(range(TA)) + [4 + j for j in range(TA)])
JLB = tuple(list(range(TA, 4)) + [4 + j for j in range(TA, 4)])

_CACHE = {}


def _build_nc():
    import concourse.mybir as mybir
    import concourse.tile as tile
    from concourse import bacc
    from contextlib import ExitStack

    F32 = mybir.dt.float32
    BF16 = mybir.dt.bfloat16
    EXP = mybir.ActivationFunctionType.Exp

    nc = bacc.Bacc("TRN2", target_bir_lowering=False, debug=False,
                   num_devices=NCORES)

    xT_d = nc.declare_dram_parameter("xT", [D, S], BF16, isOutput=False)
    encT_d = nc.declare_dram_parameter("encT", [D, S], BF16, isOutput=False)
    wqsT_d = nc.declare_dram_parameter("wqsT", [D, CPC], BF16, isOutput=False)
    wksT_d = nc.declare_dram_parameter("wksT", [D, CPC], BF16, isOutput=False)
    wvsT_d = nc.declare_dram_parameter("wvsT", [D, CPC], BF16, isOutput=False)
    wqcT_d = nc.declare_dram_parameter("wqcT", [D, CPC], BF16, isOutput=False)
    wkcT_d = nc.declare_dram_parameter("wkcT", [D, CPC], BF16, isOutput=False)
    wvcT_d = nc.declare_dram_parameter("wvcT", [D, CPC], BF16, isOutput=False)
    ident_d = nc.declare_dram_parameter("ident", [128, 128], BF16, isOutput=False)
    gmask_d = nc.declare_dram_parameter("gmask", [128, 128], BF16, isOutput=False)
    srcb_d = nc.declare_dram_parameter("srcb", [128, ST], F32, isOutput=False)
    esela_d = nc.declare_dram_parameter("esela", [NA, NA * DKH], BF16,
                                        isOutput=False)
    eselb_d = nc.declare_dram_parameter("eselb", [NB, NB * DKH], BF16,
                                        isOutput=False)
    out_d = nc.declare_dram_parameter("outT", [HPC * AUG, S], F32, isOutput=True)

    cc_in_a = nc.dram_tensor("cc_in_a", [64 * NA, S], BF16)
    cc_out_a = nc.dram_tensor("cc_out_a", [128 * NA, S], BF16)
    cc_in_b = nc.dram_tensor("cc_in_b", [64 * NB, S], BF16)
    cc_out_b = nc.dram_tensor("cc_out_b", [128 * NB, S], BF16)
    groups = [[0, 1], [2, 3], [4, 5], [6, 7]]

    def banks(lo, hi):
        res = []
        for b0 in range(0, hi, 512):
            c0, c1 = max(lo, b0), min(hi, b0 + 512)
            if c0 < c1:
                res.append((c0, c1))
        return res

    with tile.TileContext(nc) as tc:
      with nc.allow_low_precision(reason="bf16 matmuls within 2e-2 tolerance"):
        with ExitStack() as stk:
            const = stk.enter_context(tc.tile_pool(name="const", bufs=1))
            xe = stk.enter_context(tc.tile_pool(name="xe", bufs=16))
            wp = stk.enter_context(tc.tile_pool(name="wp", bufs=32))
            qk = stk.enter_context(tc.tile_pool(name="qk", bufs=12))
            vap = stk.enter_context(tc.tile_pool(name="vap", bufs=1))
            atp = stk.enter_context(tc.tile_pool(name="atp", bufs=3))
            o1p = stk.enter_context(tc.tile_pool(name="o1p", bufs=8))
            o2p = stk.enter_context(tc.tile_pool(name="o2p", bufs=3))
            dnp = stk.enter_context(tc.tile_pool(name="dnp", bufs=1))
            x1p = stk.enter_context(tc.tile_pool(name="x1p", bufs=8))
            mmps = stk.enter_context(tc.tile_pool(name="mmps", bufs=3, space="PSUM"))
            avps = stk.enter_context(tc.tile_pool(name="avps", bufs=1, space="PSUM"))

            # ---- load x^T + phase-1 weights first (two HWDGE queues in parallel) ----
            xt = []
            for j in range(NDT):
                t = xe.tile([128, S], BF16, name=f"xt{j}", tag="xe")
                nc.sync.dma_start(out=t[:], in_=xT_d[128 * j:128 * (j + 1), :])
                xt.append(t)

            def load_w(w_d, label, eng=None):
                eng = eng or nc.scalar
                ts = []
                for j in range(NDT):
                    t = wp.tile([128, CPC], BF16, name=f"w{label}{j}", tag="w")
                    eng.dma_start(out=t[:], in_=w_d[128 * j:128 * (j + 1), :])
                    ts.append(t)
                return ts

            wqs = load_w(wqsT_d, "qs")
            wks = load_w(wksT_d, "ks")
            wvs = load_w(wvsT_d, "vs")

            # ---- constants ----
            ident = const.tile([128, 128], BF16, name="ident")
            nc.sync.dma_start(out=ident[:], in_=ident_d[:])
            gmask = const.tile([128, 128], BF16, name="gmask")
            nc.sync.dma_start(out=gmask[:], in_=gmask_d[:])
            srcb = const.tile([128, ST], F32, name="srcb")
            nc.sync.dma_start(out=srcb[:], in_=srcb_d[:])
            esela = const.tile([NA, NA * DKH], BF16, name="esela")
            nc.sync.dma_start(out=esela[:], in_=esela_d[:])
            eselb = const.tile([NB, NB * DKH], BF16, name="eselb")
            nc.sync.dma_start(out=eselb[:], in_=eselb_d[:])
            ones_r8 = const.tile([128, HPC], BF16, name="ones_r8")
            nc.any.memset(ones_r8[:], 1.0)

            # ---- projection helpers (generators: one yield per 128-contraction
            # chunk, so independent matmul work can be pumped into the PE stream
            # at fine grain between attention steps — keeps the PE queue deep and
            # the HAM clock-gate warm) ----
            class _Fill:
                # Tile's RAW tracking is emission-order based: a consumer emitted
                # before its producer reads uninitialized memory. ensure() drains
                # a specific producer right before its consumer is emitted.
                def __init__(self):
                    self.q = {}

                def add(self, key, gen):
                    self.q[key] = gen

                def pump(self, n):
                    while n > 0 and self.q:
                        key = next(iter(self.q))
                        try:
                            next(self.q[key])
                            n -= 1
                        except StopIteration:
                            del self.q[key]

                def ensure(self, key):
                    gen = self.q.pop(key, None)
                    if gen is not None:
                        for _ in gen:
                            pass

                def drain(self):
                    self.pump(1 << 30)

            fill = _Fill()

            def run(gen):
                for _ in gen:
                    pass

            def gen_proj_ct(dst_tile, w_tiles, rhs, label, ct, jorder=None):
                # dst [128, S] bf16 (transposed layout), contraction over NDT chunks
                jorder = list(jorder or range(NDT))
                ps = mmps.tile([128, S], F32, name=f"ps{label}{ct}", tag="mm")
                for n, j in enumerate(jorder):
                    lhs = w_tiles[j][:, 128 * ct:128 * (ct + 1)]
                    for (c0, c1) in banks(0, S):
                        nc.tensor.matmul(ps[:, c0:c1], lhs, rhs[j][:, c0:c1],
                                         start=(n == 0), stop=(n == NDT - 1))
                    yield
                nc.vector.tensor_copy(dst_tile[:], ps[:])

            def gen_proj_v(va_tile, wv_tiles, rhs, st_, label):
                ps = mmps.tile([128, S], F32, name=f"psv{label}{st_}", tag="mm")
                for j in range(NDT):
                    nc.tensor.matmul(ps[:, 0:CPC],
                                     rhs[j][:, 128 * st_:128 * (st_ + 1)],
                                     wv_tiles[j][:, :],
                                     start=(j == 0), stop=(j == NDT - 1))
                    yield
                dst3 = va_tile[:, :].rearrange("p (h a) -> p h a", a=AUG)
                nc.vector.tensor_copy(dst3[:, :, 0:DKH],
                                      ps[:, 0:CPC].rearrange("p (h d) -> p h d", d=DKH))
                nc.vector.tensor_copy(dst3[:, :, DKH:AUG],
                                      ones_r8[:, :].rearrange("p (h o) -> p h o", o=1))

            # ---- attention (shared between phases) ----
            def attention(h, q_tiles, k_tiles, va_tiles, out_cb, causal, label,
                          pump=0, qk_keys=(), va_pfx=None):
                for key in qk_keys:
                    fill.ensure(key)
                th, ro = h // 2, 64 * (h % 2)
                qh = q_tiles[th][ro:ro + DKH, :]
                kh = k_tiles[th][ro:ro + DKH, :]
                avp_t = avps.tile([AUG, S], F32, name=f"av{label}{h}", tag="av")

                def emit_av(i, at, lo):
                    if va_pfx is not None:
                        fill.ensure(f"{va_pfx}{i}")
                    for (c0, c1) in banks(lo, S):
                        stop = (i == ST - 1) if not causal else (
                            i == min(ST - 1, (c1 - 1) // 128))
                        nc.tensor.matmul(avp_t[:, c0:c1],
                                         va_tiles[i][:, AUG * h:AUG * h + AUG],
                                         at[:, c0:c1],
                                         start=(i == 0), stop=stop)

                prev = None
                for i in range(ST):
                    lo = 128 * i if causal else 0
                    scp = mmps.tile([128, S], F32, name=f"sc{label}{h}_{i}", tag="mm")
                    for (c0, c1) in banks(lo, S):
                        diag = causal and c0 <= lo < c1
                        nc.tensor.matmul(scp[:, c0:c1],
                                         kh[:, 128 * i:128 * (i + 1)],
                                         qh[:, c0:c1], start=True, stop=not diag)
                        if diag:
                            # causal mask: accumulate -1e9 upper-triangle into the
                            # diagonal block on the PE (keeps DVE off the exp->AV
                            # critical path)
                            nc.tensor.matmul(scp[:, lo:lo + 128], ident[:, :],
                                             gmask[:, :], start=False, stop=True)
                    at = atp.tile([128, S], BF16, name=f"at{label}{h}_{i}", tag="at")
                    if causal:
                        nc.scalar.activation(at[:, lo:S], scp[:, lo:S], EXP, scale=0.125)
                    else:
                        nc.scalar.activation(at[:, :], scp[:, :], EXP,
                                             bias=srcb[:, i:i + 1], scale=0.125)
                    if pump:
                        fill.pump(pump)
                    if prev is not None:
                        emit_av(*prev)
                    prev = (i, at, lo)
                emit_av(*prev)
                out_cb(h, avp_t)

            # ---- phase 1: q/k proj interleaved with attention head pairs ----
            qt = [qk.tile([128, S], BF16, name=f"qt{ct}", tag="qk") for ct in range(4)]
            kt = [qk.tile([128, S], BF16, name=f"kt{ct}", tag="qk") for ct in range(4)]
            va = [vap.tile([128, HPC * AUG], BF16, name=f"va{st_}", tag=f"va{st_}")
                  for st_ in range(ST)]
            x1t = [x1p.tile([128, S], BF16, name=f"x1t{ct}", tag="x1") for ct in range(4)]
            # denominators packed 6/2: heads 0-5 exchange mid-attention (hidden),
            # heads 6-7 form a small final exchange
            denp_a = dnp.tile([NA, S], BF16, name="denp_a", tag="denp_a")
            denp_b = dnp.tile([NB, S], BF16, name="denp_b", tag="denp_b")
            o1s = [None] * HPC

            def self_out(h, avp_t):
                o1 = o1p.tile([AUG, S], BF16, name=f"o1_{h}", tag="o1")
                nc.vector.tensor_copy(o1[:], avp_t[:])
                # pack this head's softmax denominator row (sb2sb DMA does the
                # cross-partition move the DVE can't)
                dst = (denp_a[h:h + 1, :] if h < NA
                       else denp_b[h - NA:h - NA + 1, :])
                nc.sync.dma_start(out=dst, in_=o1[DKH:AUG, :])
                o1s[h] = o1

            def norm_head(h, rcp_t, sel, si):
                th, ro = h // 2, 64 * (h % 2)
                bc = mmps.tile([128, S], F32, name=f"bc{h}", tag="mm")
                for (c0, c1) in banks(0, S):
                    nc.tensor.matmul(bc[0:DKH, c0:c1],
                                     sel[:, DKH * si:DKH * (si + 1)],
                                     rcp_t[:, c0:c1], start=True, stop=True)
                nc.vector.tensor_mul(x1t[th][ro:ro + DKH, :], o1s[h][0:DKH, :],
                                     bc[0:DKH, :])

            def exchange(tiles, cc_in, cc_out):
                for t_, x1tile in enumerate(tiles):
                    nc.sync.dma_start(out=cc_in[128 * t_:128 * (t_ + 1), :],
                                      in_=x1tile[:])
                nc.gpsimd.collective_compute(
                    "AllGather", mybir.AluOpType.bypass,
                    ins=[cc_in[:]], outs=[cc_out[:]], replica_groups=groups)

            def gather(cc_out, jlist, x1g):
                # cc_out rows = [even-core block | odd-core block] -> global
                # feature order for both cores. NOTE: these DMAs wait on the
                # collective and block the issuing HWDGE FIFO, so they are
                # emitted only after every DMA that must not wait.
                for r, j in enumerate(jlist):
                    t = x1p.tile([128, S], BF16, name=f"x1g{j}", tag="x1")
                    nc.sync.dma_start(out=t[:], in_=cc_out[128 * r:128 * (r + 1), :])
                    x1g[j] = t

            # enc + phase-2 weights prefetch (DMAs run as slots free up)
            enct = []
            for j in range(NDT):
                t = xe.tile([128, S], BF16, name=f"enct{j}", tag="xe")
                nc.sync.dma_start(out=t[:], in_=encT_d[128 * j:128 * (j + 1), :])
                enct.append(t)
            wkc = load_w(wkcT_d, "kc")
            wvc = load_w(wvcT_d, "vc", eng=nc.sync)
            wqc = load_w(wqcT_d, "qc")
            k2t = [qk.tile([128, S], BF16, name=f"k2t{ct}", tag="qk") for ct in range(4)]
            va2 = [vap.tile([128, HPC * AUG], BF16, name=f"va2_{st_}", tag=f"va{st_}")
                   for st_ in range(ST)]
            x1g = [None] * NDT

            # upfront: just enough for head 0 to start; the rest of the
            # projections stream through the filler during attention
            run(gen_proj_ct(qt[0], wqs, xt, "q", 0))
            run(gen_proj_ct(kt[0], wks, xt, "k", 0))
            run(gen_proj_v(va[0], wvs, xt, 0, "s"))
            for st_ in range(1, ST):
                fill.add(f"vas{st_}", gen_proj_v(va[st_], wvs, xt, st_, "s"))
            for ct in range(1, 4):
                fill.add(f"qt{ct}", gen_proj_ct(qt[ct], wqs, xt, "q", ct))
                fill.add(f"kt{ct}", gen_proj_ct(kt[ct], wks, xt, "k", ct))
            for ct in range(4):
                fill.add(f"k2{ct}", gen_proj_ct(k2t[ct], wkc, enct, "k2", ct))

            def attn1(h):
                th = h // 2
                attention(h, qt, kt, va, self_out, causal=True, label="s",
                          pump=5, qk_keys=(f"qt{th}", f"kt{th}"), va_pfx="vas")

            for h in range(NA):
                attn1(h)
            rcp_a = dnp.tile([NA, S], BF16, name="rcp_a", tag="rcp_a")
            nc.vector.reciprocal(rcp_a[:], denp_a[:])
            attn1(NA)
            # first-half normalize + exchange hides under the remaining heads
            for h in range(NA):
                norm_head(h, rcp_a, esela, h)
            exchange(x1t[0:NA // 2], cc_in_a, cc_out_a)
            for h in range(NA + 1, HPC):
                attn1(h)

            # bridge: v2-proj chunks cover the small-half reciprocal + AllGather
            run(gen_proj_v(va2[0], wvc, enct, 0, "c"))
            run(gen_proj_v(va2[1], wvc, enct, 1, "c"))
            rcp_b = dnp.tile([NB, S], BF16, name="rcp_b", tag="rcp_b")
            nc.vector.reciprocal(rcp_b[:], denp_b[:])
            for h in range(NA, HPC):
                norm_head(h, rcp_b, eselb, h - NA)
            exchange(x1t[NA // 2:4], cc_in_b, cc_out_b)
            gather(cc_out_a, JLA, x1g)
            for st_ in range(2, ST):
                fill.add(f"vac{st_}", gen_proj_v(va2[st_], wvc, enct, st_, "c"))
            fill.drain()
            gather(cc_out_b, JLB, x1g)

            # ---- q2 projection from the gathered full x1 (a-half chunks first);
            # ct1-3 stream through the filler inside phase-2's ACT-bound window
            q2t = [qk.tile([128, S], BF16, name=f"q2t{ct}", tag="qk") for ct in range(4)]
            Q2ORD = JLA + JLB
            run(gen_proj_ct(q2t[0], wqc, x1g, "q2", 0, jorder=Q2ORD))
            for ct in (1, 2, 3):
                fill.add(f"q2{ct}", gen_proj_ct(q2t[ct], wqc, x1g, "q2", ct,
                                                jorder=Q2ORD))

            # ---- phase 2 attention (no mask) ----
            def cross_out(h, avp_t):
                o2 = o2p.tile([AUG, S], F32, name=f"o2_{h}", tag="o2")
                nc.vector.tensor_copy(o2[:], avp_t[:])
                nc.sync.dma_start(out=out_d[AUG * h:AUG * (h + 1), :], in_=o2[:])

            for h in range(HPC):
                th = h // 2
                attention(h, q2t, k2t, va2, cross_out, causal=False, label="c",
                          pump=2, qk_keys=(f"q2{th}", f"k2{th}"), va_pfx="vac")
            fill.drain()

    nc.compile()
    return nc


def _get_nc():
    if 'nc' not in _CACHE:
        _CACHE['nc'] = _build_nc()
    return _CACHE['nc']


def kernel(x, encoder_output, src_mask, tgt_mask,
           wq_self, wk_self, wv_self, wq_cross, wk_cross, wv_cross):
    import os
    import ml_dtypes
    from concourse.bass_utils import run_bass_kernel_spmd

    BF = ml_dtypes.bfloat16
    x = np.asarray(x, np.float32)
    enc = np.asarray(encoder_output, np.float32)
    srcm = np.asarray(src_mask)
    tgtm = np.asarray(tgt_mask)

    # host-side mask conversion: diagonal-block mask as an additive -1e9
    # upper-triangle (scoresT [sk, sq] orientation), applied on-device via
    # ident^T @ gmask accumulation
    t2 = tgtm[0, 0]  # [S, S]
    blk = (t2[0:128, 0:128] != 0).T  # [sk, sq]; same for every diagonal block
    gmask = np.where(blk, np.float32(0.0), np.float32(-1e9)).astype(BF)
    ident = np.eye(128, dtype=np.float32).astype(BF)
    sv = srcm[0, 0, 0, :]  # [S]
    srcb = np.where(sv == 0, np.float32(-1e9), np.float32(0.0))
    srcb = np.ascontiguousarray(srcb.reshape(ST, 128).T)  # [128, ST]

    # per-head selectors for the reciprocal broadcast matmuls
    esela = np.zeros((NA, NA * DKH), BF)
    for h in range(NA):
        esela[h, DKH * h:DKH * (h + 1)] = 1
    eselb = np.zeros((NB, NB * DKH), BF)
    for h in range(NB):
        eselb[h, DKH * h:DKH * (h + 1)] = 1

    def wT(w, cols):
        return np.ascontiguousarray(np.asarray(w, np.float32)[cols, :].T).astype(BF)

    in_maps = []
    for c in range(NCORES):
        b, g = divmod(c, 2)
        cols = slice(CPC * g, CPC * (g + 1))
        in_maps.append({
            "xT": np.ascontiguousarray(x[b].T).astype(BF),
            "encT": np.ascontiguousarray(enc[b].T).astype(BF),
            "wqsT": wT(wq_self, cols),
            "wksT": wT(wk_self, cols),
            "wvsT": wT(wv_self, cols),
            "wqcT": wT(wq_cross, cols),
            "wkcT": wT(wk_cross, cols),
            "wvcT": wT(wv_cross, cols),
            "ident": ident,
            "gmask": gmask,
            "srcb": srcb,
            "esela": esela,
            "eselb": eselb,
        })

    nc = _get_nc()
    trace = bool(int(os.environ.get("KERNEL_TRACE", "0")))
    res = run_bass_kernel_spmd(nc, in_maps, list(range(NCORES)), trace=trace)
    if trace:
        _CACHE['exec_time_ns'] = res.exec_time_ns
        _CACHE['mean_exec_time_ns'] = res.mean_exec_time_ns
        _CACHE['res'] = res

    out = np.empty((B, S, D), np.float32)
    for c in range(NCORES):
        b, g = divmod(c, 2)
        ot = np.asarray(res.results[c]["outT"], np.float32)  # [HPC*AUG, S]
        a3 = ot.reshape(HPC, AUG, S)
        num = a3[:, :DKH, :]                      # [h, d, s]
        den = a3[:, DKH:AUG, :]                   # [h, 1, s]
        blk = (num / den).transpose(2, 0, 1)      # [s, h, d]
        out[b, :, CPC * g:CPC * (g + 1)] = blk.reshape(S, CPC)
    return out
